# revision 6
# baseline (speedup 1.0000x reference)
"""Trainium2 Bass kernel for nn_EnsembleAdaptor: batched per-member MLP.

Per ensemble member (32 total): y = relu(x @ w1.T + b1) @ w2.T + b2
with x (512, 1024), w1 (4096, 1024), b1 (4096), w2 (1024, 4096), b2 (1024).

Sharding: pure data parallel over members - 4 members per core across 8 cores.

Precision: fp16 matmuls (fp32 PSUM) everywhere except NP8 of the 32 layer-2
k-planes, which run as e4m3 DoubleRow matmuls (2 k-planes per 512-cycle
matmul = 2x fp16 FLOP rate).  NP8 is sized so the end-to-end error stays well
under the 2e-2 gate (e4m3 on the whole matmul would give ~5%; on NP8/32 of
the contraction it scales as ~sqrt(NP8/32)).

The fp8 planes are h j-tiles 0..NP8-1: their relu writes e4m3 directly.  All
16 DoubleRow matmuls of a member run as one burst inside the L1 phase (after
j-tile 7) so the PE pays the fp16<->fp8 pipeline switch twice per member
instead of twice per o-tile.  Each burst psum is drained by a DVE
scalar_tensor_tensor into u[ot] = ps8/(SH*SW2) + b2; layer 2 is then pure
fp16 and a DVE tensor_add merges u with the fp16 psum.

Schedule: ~55 warmup matmuls on scratch SBUF ramp the PE clock while the
head DMAs land; head tiles are split across the sync/scalar/gpsimd queues;
weights prefetch several tiles ahead; y stores go on the scalar queue (a
gpsimd DGE drain is ~5us when it carries the stores).
"""

import contextlib
import ctypes
import os
import sys
import types

import numpy as np
import ml_dtypes

import concourse.bass as bass
import concourse.tile as tile
from concourse import bacc, mybir
from concourse.bass_utils import run_bass_kernel_spmd


def _install_ntff_shim():
    """Provide antenv.axon_hooks + the ctypes NTFF profile hook when the
    image's antenv lacks them, so trace=True works under axon. Safe no-op
    on failure."""
    try:
        import antenv.axon_hooks  # noqa: F401
        return
    except ImportError:
        pass
    try:
        mod = types.ModuleType("antenv.axon_hooks")
        _state = {"hook": None}
        mod.set_axon_ntff_profile_hook = lambda h: _state.__setitem__("hook", h)
        mod.get_axon_ntff_profile_hook = lambda: _state["hook"]
        sys.modules["antenv.axon_hooks"] = mod
        import antenv
        antenv.axon_hooks = mod

        so_path = "/opt/axon/libaxon_pjrt.so"
        if not os.path.exists(so_path):
            return
        lib = ctypes.CDLL(so_path)
        if not hasattr(lib, "axon_start_nrt_profile"):
            return
        lib.axon_start_nrt_profile.argtypes = [
            ctypes.POINTER(ctypes.c_int64),
            ctypes.c_size_t,
        ]
        lib.axon_start_nrt_profile.restype = ctypes.c_int64
        lib.axon_stop_nrt_profile.argtypes = [ctypes.c_char_p]
        lib.axon_stop_nrt_profile.restype = ctypes.c_int64

        @contextlib.contextmanager
        def _hook(output_dir, device_ids):
            import jax
            jax.devices()
            if device_ids:
                ids = (ctypes.c_int64 * len(device_ids))(*device_ids)
                rc = lib.axon_start_nrt_profile(ids, len(device_ids))
            else:
                rc = lib.axon_start_nrt_profile(None, 0)
            if rc != 0:
                raise RuntimeError(f"axon_start_nrt_profile rc={rc}")
            try:
                yield
            finally:
                n = lib.axon_stop_nrt_profile(str(output_dir).encode())
                print(f"profile: {n} file(s) written to {output_dir}",
                      file=sys.stderr)

        mod.set_axon_ntff_profile_hook(_hook)
    except Exception:
        pass

B, S, DIN, H, DOUT = 32, 512, 1024, 4096, 1024
N_W1 = H * DIN
N_B1 = H
N_W2 = DOUT * H
N_B2 = DOUT

N_CORES = 8
M_PER = B // N_CORES  # members per core

DT = DIN // 128   # 8  k-planes for layer 1
JT = H // 128     # 32 j-tiles (layer-1 outputs / layer-2 k-planes)
OT = DOUT // 128  # 8  o-tiles for layer 2
SN = S            # 512 moving free dim

NP8 = 4           # layer-2 k-planes computed in e4m3 DoubleRow (even)
KF16 = JT - NP8   # layer-2 k-planes kept fp16 (j-tiles NP8..31)

F16 = mybir.dt.float16
FP8 = mybir.dt.float8e4
F32 = mybir.dt.float32
NP_F16 = np.float16
NP_FP8 = ml_dtypes.float8_e4m3  # TRN fp8e4: max 240

SH = 32.0         # scale on the fp8 copy of h   (sigma ~.45 -> 14)
SW2 = 1024.0      # scale on the fp8 w2 planes   (sigma .02 -> 20)
ALPHA2 = 1.0 / (SH * SW2)

N_WARM = 55       # PE warmup matmuls on scratch SBUF (clock ramp + head DMA)
WARM_ROWS = 48

_cache = {}

DR = mybir.MatmulPerfMode.DoubleRow


def _build_nc():
    nc = bacc.Bacc("TRN2", target_bir_lowering=False, enable_partition_id=False)
    xp = nc.dram_tensor("xp", [M_PER, 128, DT, SN], F16, kind="ExternalInput")
    w1p = nc.dram_tensor("w1p", [M_PER, JT, 128, DT, 128], F16,
                         kind="ExternalInput")
    w2p16 = nc.dram_tensor("w2p16", [M_PER, OT, 128, KF16, 128], F16,
                           kind="ExternalInput")
    w2p8 = nc.dram_tensor("w2p8", [M_PER, 128, OT, NP8, 128], FP8,
                          kind="ExternalInput")
    b1p16 = nc.dram_tensor("b1p16", [M_PER, 128, KF16], F32,
                           kind="ExternalInput")
    b1p8 = nc.dram_tensor("b1p8", [M_PER, 128, NP8], F32, kind="ExternalInput")
    b2p = nc.dram_tensor("b2p", [M_PER, 128, OT], F32, kind="ExternalInput")
    ytp = nc.dram_tensor("ytp", [M_PER, OT, 128, SN], F32, kind="ExternalOutput")

    relu = mybir.ActivationFunctionType.Relu

    with tile.TileContext(nc) as tc:
        with (
            tc.tile_pool(name="xpool", bufs=2) as xpool,
            tc.tile_pool(name="w1pool", bufs=6) as w1pool,
            tc.tile_pool(name="w2pool16", bufs=4) as w2pool16,
            tc.tile_pool(name="w2pool8", bufs=2) as w2pool8,
            tc.tile_pool(name="bpool", bufs=2) as bpool,
            tc.tile_pool(name="h16pool", bufs=2) as h16pool,
            tc.tile_pool(name="h8pool", bufs=2) as h8pool,
            tc.tile_pool(name="upool", bufs=10) as upool,
            tc.tile_pool(name="ypool", bufs=4) as ypool,
            tc.tile_pool(name="scratch", bufs=1) as scratch,
            tc.tile_pool(name="ps1", bufs=2, space="PSUM") as ps1pool,
            tc.tile_pool(name="ps2", bufs=2, space="PSUM") as ps2pool,
            tc.tile_pool(name="ps8", bufs=3, space="PSUM") as ps8pool,
            tc.tile_pool(name="pswarm", bufs=1, space="PSUM") as pswarmpool,
        ):
            # ---- PE warmup: ramp the tensor-engine clock while head DMAs
            # land.  Scratch SBUF is memset on gpsimd (its queue starts as
            # early as the PE's); the PSUM result is never read.
            sw = scratch.tile([128, 128], F16)
            sx = scratch.tile([128, WARM_ROWS], F16)
            nc.gpsimd.memset(sw[:], 0)
            nc.gpsimd.memset(sx[:], 0)
            psw = pswarmpool.tile([128, WARM_ROWS], F32)
            for _ in range(N_WARM):
                nc.tensor.matmul(psw[:], sw[:], sx[:], start=True, stop=True)

            # ---- DMA issue helpers (tiles keyed for later consumption) ----
            t_x, t_w1, t_w2, t_w28, t_b = {}, {}, {}, {}, {}

            def issue_x(m):
                x_t = xpool.tile([128, DT, SN], F16)
                if m == 0:
                    nc.scalar.dma_start(x_t[:, 0:1, :], xp[m, :, 0:1, :])
                    nc.scalar.dma_start(x_t[:, 1:3, :], xp[m, :, 1:3, :])
                    nc.scalar.dma_start(x_t[:, 3:, :], xp[m, :, 3:, :])
                else:
                    nc.scalar.dma_start(x_t[:], xp[m])
                t_x[m] = x_t

            def issue_w1(m, jt, queue=None):
                w1_t = w1pool.tile([128, DT, 128], F16, tag="w1_t")
                q = queue or nc.sync
                if m == 0 and jt == 0:
                    q.dma_start(w1_t[:, 0:2, :], w1p[m, jt, :, 0:2, :])
                    q.dma_start(w1_t[:, 2:, :], w1p[m, jt, :, 2:, :])
                else:
                    q.dma_start(w1_t[:], w1p[m, jt])
                t_w1[(m, jt)] = w1_t

            def issue_w2(m, ot):
                w16_t = w2pool16.tile([128, KF16, 128], F16, tag="w2_16")
                nc.sync.dma_start(w16_t[:], w2p16[m, ot])
                t_w2[(m, ot)] = w16_t

            def issue_w28(m):
                w8_t = w2pool8.tile([128, OT, NP8, 128], FP8)
                nc.gpsimd.dma_start(w8_t[:], w2p8[m])
                t_w28[m] = w8_t

            def issue_b(m):
                b1_t = bpool.tile([128, KF16], F32, tag="b1")
                nc.gpsimd.dma_start(b1_t[:], b1p16[m])
                b1s_t = bpool.tile([128, NP8], F32, tag="b1s")
                nc.gpsimd.dma_start(b1s_t[:], b1p8[m])
                b2_t = bpool.tile([128, OT], F32, tag="b2")
                nc.gpsimd.dma_start(b2_t[:], b2p[m])
                t_b[m] = (b1_t, b1s_t, b2_t)

            # ---- head loads, split across queues ----
            issue_w1(0, 0)            # sync
            issue_x(0)                # scalar
            issue_b(0)                # gpsimd
            issue_w28(0)              # gpsimd
            issue_w1(0, 1, queue=nc.gpsimd)
            issue_w1(0, 2)            # sync
            issue_w1(0, 3)            # sync

            W1_PREF = 4   # w1 prefetch depth (j-tiles ahead)
            W2_PREF = 2   # w2p16 prefetch depth (o-tiles ahead)

            for m in range(M_PER):
                x_t = t_x[m]
                b1_t, b1s_t, b2_t = t_b[m]
                w28_t = t_w28[m]
                h16_t = h16pool.tile([128, KF16, SN], F16)
                h8_t = h8pool.tile([128, NP8, SN], FP8)
                u_ts = []

                for jt in range(JT):
                    if jt + W1_PREF < JT and (m, jt + W1_PREF) not in t_w1:
                        issue_w1(m, jt + W1_PREF)
                    if jt == 20:
                        issue_w2(m, 0)
                    if jt == 24:
                        issue_w2(m, 1)
                    if jt == 26 and m + 1 < M_PER:
                        issue_x(m + 1)
                        issue_b(m + 1)
                        issue_w28(m + 1)
                    w1_t = t_w1.pop((m, jt))
                    ps = ps1pool.tile([128, SN], F32)
                    for k in range(DT):
                        nc.tensor.matmul(ps[:], w1_t[:, k, :], x_t[:, k, :],
                                         start=(k == 0), stop=(k == DT - 1))
                    if jt < NP8:
                        nc.scalar.activation(h8_t[:, jt, :], ps[:], relu,
                                             bias=b1s_t[:, jt:jt + 1],
                                             scale=SH)
                    else:
                        nc.scalar.activation(h16_t[:, jt - NP8, :], ps[:],
                                             relu, bias=b1_t[:, jt - NP8:
                                                             jt - NP8 + 1])

                    if jt == DT - 1:
                        # ---- fp8 DoubleRow burst: all layer-2 fp8 partial
                        # sums for this member, one pipeline-mode switch pair.
                        for ot in range(OT):
                            ps8t = ps8pool.tile([128, SN], F32, tag="ps8")
                            for p in range(NP8 // 2):
                                nc.tensor.matmul(
                                    ps8t[:],
                                    w28_t[:, ot, 2 * p:2 * p + 2, :],
                                    h8_t[:, 2 * p:2 * p + 2, :],
                                    start=(p == 0), stop=(p == NP8 // 2 - 1),
                                    perf_mode=DR)
                            u_t = upool.tile([128, SN], F32, tag="u_t")
                            nc.vector.scalar_tensor_tensor(
                                u_t[:], ps8t[:], ALPHA2,
                                b2_t[:, ot:ot + 1].broadcast_to([128, SN]),
                                op0=mybir.AluOpType.mult,
                                op1=mybir.AluOpType.add)
                            u_ts.append(u_t)

                for ot in range(OT):
                    if ot + W2_PREF < OT:
                        issue_w2(m, ot + W2_PREF)
                    if m + 1 < M_PER and ot >= OT - W1_PREF:
                        issue_w1(m + 1, ot - (OT - W1_PREF))
                    w16_t = t_w2.pop((m, ot))
                    u_t = u_ts[ot]
                    # Last o-tile of the last member: quarter the moving dim
                    # so the add/store tail overlaps the remaining matmuls.
                    quarters = (
                        [(q * (SN // 4), (q + 1) * (SN // 4)) for q in range(4)]
                        if (m == M_PER - 1 and ot == OT - 1) else [(0, SN)]
                    )
                    for lo, hi in quarters:
                        w = hi - lo
                        ps2t = ps2pool.tile([128, w], F32, tag="ps2")
                        for k in range(KF16):
                            nc.tensor.matmul(ps2t[:], w16_t[:, k, :],
                                             h16_t[:, k, lo:hi],
                                             start=(k == 0),
                                             stop=(k == KF16 - 1))
                        y_t = ypool.tile([128, w], F32, tag="y_t")
                        nc.vector.tensor_add(y_t[:], u_t[:, lo:hi], ps2t[:])
                        nc.scalar.dma_start(ytp[m, ot, :, lo:hi], y_t[:])
    nc.compile()
    return nc


def _pack_core(x_flat, ensemble_weights, members):
    """Pack one core's members into the DMA-friendly device layouts."""
    n = len(members)
    xp = np.empty((n, 128, DT, SN), dtype=NP_F16)
    w1p = np.empty((n, JT, 128, DT, 128), dtype=NP_F16)
    w2p16 = np.empty((n, OT, 128, KF16, 128), dtype=NP_F16)
    w2p8 = np.empty((n, 128, OT, NP8, 128), dtype=NP_FP8)
    b1p16 = np.empty((n, 128, KF16), dtype=np.float32)
    b1p8 = np.empty((n, 128, NP8), dtype=np.float32)
    b2p = np.empty((n, 128, OT), dtype=np.float32)
    for i, mem in enumerate(members):
        x = x_flat[mem].reshape(S, DIN)
        o = 0
        w1 = ensemble_weights[mem, o:o + N_W1].reshape(H, DIN); o += N_W1
        b1 = ensemble_weights[mem, o:o + N_B1]; o += N_B1
        w2 = ensemble_weights[mem, o:o + N_W2].reshape(DOUT, H); o += N_W2
        b2 = ensemble_weights[mem, o:o + N_B2]
        # xp[p, t, s] = x[s, t*128+p]
        xp[i] = x.reshape(S, DT, 128).transpose(2, 1, 0).astype(NP_F16)
        # w1p[jt, p, t, jj] = w1[jt*128+jj, t*128+p]
        w1p[i] = (w1.reshape(JT, 128, DT, 128).transpose(0, 3, 2, 1)
                  .astype(NP_F16))
        # w2 planes: t = layer-2 contraction plane (h j-plane); planes
        # 0..NP8-1 are the fp8 ones.
        # w2v[ot, p, t, oo] = w2[ot*128+oo, t*128+p]
        w2v = w2.reshape(OT, 128, JT, 128).transpose(0, 3, 2, 1)
        w2p16[i] = w2v[:, :, NP8:].astype(NP_F16)
        w2p8[i] = (np.clip(w2v[:, :, :NP8] * SW2, -240.0, 240.0)
                   .astype(NP_FP8).transpose(1, 0, 2, 3))
        b1t = b1.reshape(JT, 128).T.astype(np.float32)  # [128, JT]
        b1p16[i] = b1t[:, NP8:]
        b1p8[i] = b1t[:, :NP8] * SH
        b2p[i] = b2.reshape(OT, 128).T.astype(np.float32)
    return {"xp": xp, "w1p": w1p, "w2p16": w2p16, "w2p8": w2p8,
            "b1p16": b1p16, "b1p8": b1p8, "b2p": b2p}


def kernel(x_flat: np.ndarray, ensemble_weights: np.ndarray) -> np.ndarray:
    x_flat = np.asarray(x_flat, dtype=np.float32)
    ensemble_weights = np.asarray(ensemble_weights, dtype=np.float32)

    if "nc" not in _cache:
        _cache["nc"] = _build_nc()
    nc = _cache["nc"]

    in_maps = [
        _pack_core(x_flat, ensemble_weights,
                   list(range(c * M_PER, (c + 1) * M_PER)))
        for c in range(N_CORES)
    ]

    trace = bool(int(os.environ.get("KERNEL_TRACE", "0")))
    if trace:
        _install_ntff_shim()
    res = run_bass_kernel_spmd(nc, in_maps, core_ids=list(range(N_CORES)),
                               trace=trace)
    if trace:
        _cache["exec_time_ns"] = res.exec_time_ns

    out = np.empty((B, S * DOUT), dtype=np.float32)
    for c in range(N_CORES):
        ytp = res.results[c]["ytp"]  # (M_PER, OT, 128, SN)
        for i in range(M_PER):
            mem = c * M_PER + i
            # y[s, ot*128+p] = ytp[i, ot, p, s]
            out[mem] = (
                ytp[i].transpose(2, 0, 1).reshape(S * DOUT).astype(np.float32)
            )
    return out


# revision 7
# speedup vs baseline: 1.0107x; 1.0107x over previous
"""Trainium2 Bass kernel for nn_EnsembleAdaptor: batched per-member MLP.

Per ensemble member (32 total): y = relu(x @ w1.T + b1) @ w2.T + b2
with x (512, 1024), w1 (4096, 1024), b1 (4096), w2 (1024, 4096), b2 (1024).

Sharding: pure data parallel over members - 4 members per core across 8 cores.

Precision: fp16 matmuls (fp32 PSUM) everywhere except NP8 of the 32 layer-2
k-planes, which run as e4m3 DoubleRow matmuls (2 k-planes per 512-cycle
matmul = 2x fp16 FLOP rate).  NP8 is sized so the end-to-end error stays well
under the 2e-2 gate (e4m3 on the whole matmul would give ~5%; on NP8/32 of
the contraction it scales as ~sqrt(NP8/32)).

The fp8 planes are h j-tiles 0..NP8-1: their relu writes e4m3 directly.  All
16 DoubleRow matmuls of a member run as one burst inside the L1 phase (after
j-tile 7) so the PE pays the fp16<->fp8 pipeline switch twice per member
instead of twice per o-tile.  Each burst psum is drained by a DVE
scalar_tensor_tensor into u[ot] = ps8/(SH*SW2) + b2; layer 2 is then pure
fp16 and a DVE tensor_add merges u with the fp16 psum.

Schedule: ~55 warmup matmuls on scratch SBUF ramp the PE clock while the
head DMAs land; head tiles are split across the sync/scalar/gpsimd queues;
weights prefetch several tiles ahead; y stores go on the scalar queue (a
gpsimd DGE drain is ~5us when it carries the stores).
"""

import contextlib
import ctypes
import os
import sys
import types

import numpy as np
import ml_dtypes

import concourse.bass as bass
import concourse.tile as tile
from concourse import bacc, mybir
from concourse.bass_utils import run_bass_kernel_spmd


def _install_ntff_shim():
    """Provide antenv.axon_hooks + the ctypes NTFF profile hook when the
    image's antenv lacks them, so trace=True works under axon. Safe no-op
    on failure."""
    try:
        import antenv.axon_hooks  # noqa: F401
        return
    except ImportError:
        pass
    try:
        mod = types.ModuleType("antenv.axon_hooks")
        _state = {"hook": None}
        mod.set_axon_ntff_profile_hook = lambda h: _state.__setitem__("hook", h)
        mod.get_axon_ntff_profile_hook = lambda: _state["hook"]
        sys.modules["antenv.axon_hooks"] = mod
        import antenv
        antenv.axon_hooks = mod

        so_path = "/opt/axon/libaxon_pjrt.so"
        if not os.path.exists(so_path):
            return
        lib = ctypes.CDLL(so_path)
        if not hasattr(lib, "axon_start_nrt_profile"):
            return
        lib.axon_start_nrt_profile.argtypes = [
            ctypes.POINTER(ctypes.c_int64),
            ctypes.c_size_t,
        ]
        lib.axon_start_nrt_profile.restype = ctypes.c_int64
        lib.axon_stop_nrt_profile.argtypes = [ctypes.c_char_p]
        lib.axon_stop_nrt_profile.restype = ctypes.c_int64

        @contextlib.contextmanager
        def _hook(output_dir, device_ids):
            import jax
            jax.devices()
            if device_ids:
                ids = (ctypes.c_int64 * len(device_ids))(*device_ids)
                rc = lib.axon_start_nrt_profile(ids, len(device_ids))
            else:
                rc = lib.axon_start_nrt_profile(None, 0)
            if rc != 0:
                raise RuntimeError(f"axon_start_nrt_profile rc={rc}")
            try:
                yield
            finally:
                n = lib.axon_stop_nrt_profile(str(output_dir).encode())
                print(f"profile: {n} file(s) written to {output_dir}",
                      file=sys.stderr)

        mod.set_axon_ntff_profile_hook(_hook)
    except Exception:
        pass

B, S, DIN, H, DOUT = 32, 512, 1024, 4096, 1024
N_W1 = H * DIN
N_B1 = H
N_W2 = DOUT * H
N_B2 = DOUT

N_CORES = 8
M_PER = B // N_CORES  # members per core

DT = DIN // 128   # 8  k-planes for layer 1
JT = H // 128     # 32 j-tiles (layer-1 outputs / layer-2 k-planes)
OT = DOUT // 128  # 8  o-tiles for layer 2
SN = S            # 512 moving free dim

NP8 = 4           # layer-2 k-planes computed in e4m3 DoubleRow (even)
KF16 = JT - NP8   # layer-2 k-planes kept fp16 (j-tiles NP8..31)

F16 = mybir.dt.float16
FP8 = mybir.dt.float8e4
F32 = mybir.dt.float32
NP_F16 = np.float16
NP_FP8 = ml_dtypes.float8_e4m3  # TRN fp8e4: max 240

SH = 32.0         # scale on the fp8 copy of h   (sigma ~.45 -> 14)
SW2 = 1024.0      # scale on the fp8 w2 planes   (sigma .02 -> 20)
ALPHA2 = 1.0 / (SH * SW2)

N_WARM = 100      # PE warmup matmuls on scratch SBUF (clock ramp + head DMA)
WARM_ROWS = 48

_cache = {}

DR = mybir.MatmulPerfMode.DoubleRow


def _build_nc():
    nc = bacc.Bacc("TRN2", target_bir_lowering=False, enable_partition_id=False)
    xp = nc.dram_tensor("xp", [M_PER, 128, DT, SN], F16, kind="ExternalInput")
    w1p = nc.dram_tensor("w1p", [M_PER, JT, 128, DT, 128], F16,
                         kind="ExternalInput")
    w2p16 = nc.dram_tensor("w2p16", [M_PER, OT, 128, KF16, 128], F16,
                           kind="ExternalInput")
    w2p8 = nc.dram_tensor("w2p8", [M_PER, 128, OT, NP8, 128], FP8,
                          kind="ExternalInput")
    b1p16 = nc.dram_tensor("b1p16", [M_PER, 128, KF16], F32,
                           kind="ExternalInput")
    b1p8 = nc.dram_tensor("b1p8", [M_PER, 128, NP8], F32, kind="ExternalInput")
    b2p = nc.dram_tensor("b2p", [M_PER, 128, OT], F32, kind="ExternalInput")
    ytp = nc.dram_tensor("ytp", [M_PER, OT, 128, SN], F32, kind="ExternalOutput")

    relu = mybir.ActivationFunctionType.Relu

    with tile.TileContext(nc) as tc:
        with (
            tc.tile_pool(name="xpool", bufs=2) as xpool,
            tc.tile_pool(name="w1pool", bufs=6) as w1pool,
            tc.tile_pool(name="w2pool16", bufs=4) as w2pool16,
            tc.tile_pool(name="w2pool8", bufs=2) as w2pool8,
            tc.tile_pool(name="bpool", bufs=2) as bpool,
            tc.tile_pool(name="h16pool", bufs=2) as h16pool,
            tc.tile_pool(name="h8pool", bufs=2) as h8pool,
            tc.tile_pool(name="upool", bufs=10) as upool,
            tc.tile_pool(name="ypool", bufs=4) as ypool,
            tc.tile_pool(name="scratch", bufs=1) as scratch,
            tc.tile_pool(name="ps1", bufs=3, space="PSUM") as ps1pool,
            tc.tile_pool(name="ps2", bufs=2, space="PSUM") as ps2pool,
            tc.tile_pool(name="ps8", bufs=3, space="PSUM") as ps8pool,
        ):
            # ---- PE warmup: ramp the tensor-engine clock while head DMAs
            # land.  Scratch SBUF is memset on gpsimd (its queue starts as
            # early as the PE's); the PSUM result is never read.
            sw = scratch.tile([128, 128], F16)
            sx = scratch.tile([128, WARM_ROWS], F16)
            nc.gpsimd.memset(sw[:], 0)
            nc.gpsimd.memset(sx[:], 0)
            psw = ps2pool.tile([128, WARM_ROWS], F32, tag="ps2")
            for _ in range(N_WARM):
                nc.tensor.matmul(psw[:], sw[:], sx[:], start=True, stop=True)

            # ---- DMA issue helpers (tiles keyed for later consumption) ----
            t_x, t_w1, t_w2, t_w28, t_b = {}, {}, {}, {}, {}

            def issue_x(m):
                x_t = xpool.tile([128, DT, SN], F16)
                if m == 0:
                    nc.scalar.dma_start(x_t[:, 0:2, :], xp[m, :, 0:2, :])
                    nc.scalar.dma_start(x_t[:, 2:4, :], xp[m, :, 2:4, :])
                    nc.gpsimd.dma_start(x_t[:, 4:6, :], xp[m, :, 4:6, :])
                    nc.gpsimd.dma_start(x_t[:, 6:, :], xp[m, :, 6:, :])
                else:
                    nc.scalar.dma_start(x_t[:], xp[m])
                t_x[m] = x_t

            def issue_w1(m, jt, queue=None):
                w1_t = w1pool.tile([128, DT, 128], F16, tag="w1_t")
                q = queue or nc.sync
                if m == 0 and jt == 0:
                    q.dma_start(w1_t[:, 0:2, :], w1p[m, jt, :, 0:2, :])
                    q.dma_start(w1_t[:, 2:, :], w1p[m, jt, :, 2:, :])
                else:
                    q.dma_start(w1_t[:], w1p[m, jt])
                t_w1[(m, jt)] = w1_t

            def issue_w2(m, ot):
                w16_t = w2pool16.tile([128, KF16, 128], F16, tag="w2_16")
                nc.sync.dma_start(w16_t[:], w2p16[m, ot])
                t_w2[(m, ot)] = w16_t

            def issue_w28(m):
                w8_t = w2pool8.tile([128, OT, NP8, 128], FP8)
                nc.gpsimd.dma_start(w8_t[:], w2p8[m])
                t_w28[m] = w8_t

            def issue_b(m):
                b1_t = bpool.tile([128, KF16], F32, tag="b1")
                nc.gpsimd.dma_start(b1_t[:], b1p16[m])
                b1s_t = bpool.tile([128, NP8], F32, tag="b1s")
                nc.gpsimd.dma_start(b1s_t[:], b1p8[m])
                b2_t = bpool.tile([128, OT], F32, tag="b2")
                nc.gpsimd.dma_start(b2_t[:], b2p[m])
                t_b[m] = (b1_t, b1s_t, b2_t)

            # ---- head loads, split across queues ----
            issue_w1(0, 0)            # sync
            issue_x(0)                # scalar + gpsimd
            issue_w1(0, 1)            # sync
            issue_b(0)                # gpsimd
            issue_w28(0)              # gpsimd
            issue_w1(0, 2)            # sync
            issue_w1(0, 3)            # sync

            W1_PREF = 4   # w1 prefetch depth (j-tiles ahead)
            W2_PREF = 2   # w2p16 prefetch depth (o-tiles ahead)

            for m in range(M_PER):
                x_t = t_x[m]
                b1_t, b1s_t, b2_t = t_b[m]
                w28_t = t_w28[m]
                h16_t = h16pool.tile([128, KF16, SN], F16)
                h8_t = h8pool.tile([128, NP8, SN], FP8)
                u_ts = []

                for jt in range(JT):
                    if jt + W1_PREF < JT and (m, jt + W1_PREF) not in t_w1:
                        issue_w1(m, jt + W1_PREF)
                    if jt == 20:
                        issue_w2(m, 0)
                    if jt == 24:
                        issue_w2(m, 1)
                    if jt == 26 and m + 1 < M_PER:
                        issue_x(m + 1)
                        issue_b(m + 1)
                        issue_w28(m + 1)
                    w1_t = t_w1.pop((m, jt))
                    ps = ps1pool.tile([128, SN], F32)
                    for k in range(DT):
                        nc.tensor.matmul(ps[:], w1_t[:, k, :], x_t[:, k, :],
                                         start=(k == 0), stop=(k == DT - 1))
                    if jt < NP8:
                        nc.scalar.activation(h8_t[:, jt, :], ps[:], relu,
                                             bias=b1s_t[:, jt:jt + 1],
                                             scale=SH)
                    else:
                        nc.scalar.activation(h16_t[:, jt - NP8, :], ps[:],
                                             relu, bias=b1_t[:, jt - NP8:
                                                             jt - NP8 + 1])

                    if jt == DT - 1:
                        # ---- fp8 DoubleRow burst: all layer-2 fp8 partial
                        # sums for this member, one pipeline-mode switch pair.
                        for ot in range(OT):
                            ps8t = ps8pool.tile([128, SN], F32, tag="ps8")
                            for p in range(NP8 // 2):
                                nc.tensor.matmul(
                                    ps8t[:],
                                    w28_t[:, ot, 2 * p:2 * p + 2, :],
                                    h8_t[:, 2 * p:2 * p + 2, :],
                                    start=(p == 0), stop=(p == NP8 // 2 - 1),
                                    perf_mode=DR)
                            u_t = upool.tile([128, SN], F32, tag="u_t")
                            if ot % 2 == 0:
                                nc.vector.tensor_scalar(
                                    u_t[:], ps8t[:], ALPHA2,
                                    b2_t[:, ot:ot + 1],
                                    op0=mybir.AluOpType.mult,
                                    op1=mybir.AluOpType.add)
                            else:
                                nc.scalar.activation(
                                    u_t[:], ps8t[:],
                                    mybir.ActivationFunctionType.Identity,
                                    bias=b2_t[:, ot:ot + 1], scale=ALPHA2)
                            u_ts.append(u_t)

                for ot in range(OT):
                    if ot + W2_PREF < OT:
                        issue_w2(m, ot + W2_PREF)
                    if m + 1 < M_PER and ot >= OT - W1_PREF:
                        issue_w1(m + 1, ot - (OT - W1_PREF))
                    w16_t = t_w2.pop((m, ot))
                    u_t = u_ts[ot]
                    # Last o-tile of the last member: quarter the moving dim
                    # so the add/store tail overlaps the remaining matmuls.
                    quarters = (
                        [(q * (SN // 4), (q + 1) * (SN // 4)) for q in range(4)]
                        if (m == M_PER - 1 and ot == OT - 1) else [(0, SN)]
                    )
                    for lo, hi in quarters:
                        w = hi - lo
                        ps2t = ps2pool.tile([128, w], F32, tag="ps2")
                        for k in range(KF16):
                            nc.tensor.matmul(ps2t[:], w16_t[:, k, :],
                                             h16_t[:, k, lo:hi],
                                             start=(k == 0),
                                             stop=(k == KF16 - 1))
                        y_t = ypool.tile([128, w], F32, tag="y_t")
                        nc.vector.tensor_add(y_t[:], u_t[:, lo:hi], ps2t[:])
                        nc.sync.dma_start(ytp[m, ot, :, lo:hi], y_t[:])
    nc.compile()
    return nc


def _pack_core(x_flat, ensemble_weights, members):
    """Pack one core's members into the DMA-friendly device layouts."""
    n = len(members)
    xp = np.empty((n, 128, DT, SN), dtype=NP_F16)
    w1p = np.empty((n, JT, 128, DT, 128), dtype=NP_F16)
    w2p16 = np.empty((n, OT, 128, KF16, 128), dtype=NP_F16)
    w2p8 = np.empty((n, 128, OT, NP8, 128), dtype=NP_FP8)
    b1p16 = np.empty((n, 128, KF16), dtype=np.float32)
    b1p8 = np.empty((n, 128, NP8), dtype=np.float32)
    b2p = np.empty((n, 128, OT), dtype=np.float32)
    for i, mem in enumerate(members):
        x = x_flat[mem].reshape(S, DIN)
        o = 0
        w1 = ensemble_weights[mem, o:o + N_W1].reshape(H, DIN); o += N_W1
        b1 = ensemble_weights[mem, o:o + N_B1]; o += N_B1
        w2 = ensemble_weights[mem, o:o + N_W2].reshape(DOUT, H); o += N_W2
        b2 = ensemble_weights[mem, o:o + N_B2]
        # xp[p, t, s] = x[s, t*128+p]
        xp[i] = x.reshape(S, DT, 128).transpose(2, 1, 0).astype(NP_F16)
        # w1p[jt, p, t, jj] = w1[jt*128+jj, t*128+p]
        w1p[i] = (w1.reshape(JT, 128, DT, 128).transpose(0, 3, 2, 1)
                  .astype(NP_F16))
        # w2 planes: t = layer-2 contraction plane (h j-plane); planes
        # 0..NP8-1 are the fp8 ones.
        # w2v[ot, p, t, oo] = w2[ot*128+oo, t*128+p]
        w2v = w2.reshape(OT, 128, JT, 128).transpose(0, 3, 2, 1)
        w2p16[i] = w2v[:, :, NP8:].astype(NP_F16)
        w2p8[i] = (np.clip(w2v[:, :, :NP8] * SW2, -240.0, 240.0)
                   .astype(NP_FP8).transpose(1, 0, 2, 3))
        b1t = b1.reshape(JT, 128).T.astype(np.float32)  # [128, JT]
        b1p16[i] = b1t[:, NP8:]
        b1p8[i] = b1t[:, :NP8] * SH
        b2p[i] = b2.reshape(OT, 128).T.astype(np.float32)
    return {"xp": xp, "w1p": w1p, "w2p16": w2p16, "w2p8": w2p8,
            "b1p16": b1p16, "b1p8": b1p8, "b2p": b2p}


def kernel(x_flat: np.ndarray, ensemble_weights: np.ndarray) -> np.ndarray:
    x_flat = np.asarray(x_flat, dtype=np.float32)
    ensemble_weights = np.asarray(ensemble_weights, dtype=np.float32)

    if "nc" not in _cache:
        _cache["nc"] = _build_nc()
    nc = _cache["nc"]

    in_maps = [
        _pack_core(x_flat, ensemble_weights,
                   list(range(c * M_PER, (c + 1) * M_PER)))
        for c in range(N_CORES)
    ]

    trace = bool(int(os.environ.get("KERNEL_TRACE", "0")))
    if trace:
        _install_ntff_shim()
    res = run_bass_kernel_spmd(nc, in_maps, core_ids=list(range(N_CORES)),
                               trace=trace)
    if trace:
        _cache["exec_time_ns"] = res.exec_time_ns

    out = np.empty((B, S * DOUT), dtype=np.float32)
    for c in range(N_CORES):
        ytp = res.results[c]["ytp"]  # (M_PER, OT, 128, SN)
        for i in range(M_PER):
            mem = c * M_PER + i
            # y[s, ot*128+p] = ytp[i, ot, p, s]
            out[mem] = (
                ytp[i].transpose(2, 0, 1).reshape(S * DOUT).astype(np.float32)
            )
    return out
